# revision 1
# baseline (speedup 1.0000x reference)
# Grouped GRU layer on 8 Trainium2 NeuronCores (one group per core).
#
# Problem: x [64, 500, 1024], 8 independent groups of (IG=128 -> HG=128) GRUs.
#   xp = einsum('btgi,gji->btgj', xg, W_ih) + b_ih        (input projection)
#   per step: hp = h @ W_hh[g].T + b_hh
#             r = sig(xr+hr); z = sig(xz+hz); n = tanh(xn + r*hn)
#             h = (1-z)*n + z*h
#
# Sharding: group g -> core g. Per-core layout is fully "transposed":
#   state h^T [HG=128 partitions, B=64 free], weights pre-transposed on host.
# Input projection matmuls write PSUM banks; the recurrence r/z matmuls
# accumulate on top (start=False), so xr+hr / xz+hz come out of PE for free.
# Sigmoid biases are folded in via the ACT per-partition bias operand, n-gate
# biases via scalar_tensor_tensor's per-partition scalar.

import numpy as np

B, T, IN, HID, G = 64, 500, 1024, 1024, 8
IG, HG = 128, 128

PSUM_STEPS = 8          # recurrence steps per PSUM bank chunk ([128, 8*64] fp32 = 1 bank)
RING_STEPS = 50         # output ring buffer length (steps) per DMA-out chunk

_CACHE = {}


def _build_program():
    import concourse.tile as tile
    from concourse import bacc, mybir

    f32 = mybir.dt.float32
    AF = mybir.ActivationFunctionType
    ALU = mybir.AluOpType

    nc = bacc.Bacc()
    xT = nc.declare_dram_parameter("xT", [IG, T * B], f32, isOutput=False)
    wih = nc.declare_dram_parameter("wih", [IG, 3 * HG], f32, isOutput=False)
    whh = nc.declare_dram_parameter("whh", [HG, 3 * HG], f32, isOutput=False)
    # per-partition bias columns: [r_bias, z_bias, b_ihn, b_hhn]
    biases = nc.declare_dram_parameter("biases", [HG, 4], f32, isOutput=False)
    y = nc.declare_dram_parameter("y", [HG, T * B], f32, isOutput=True)

    from contextlib import ExitStack

    with tile.TileContext(nc) as tc, ExitStack() as ctx:
        consts = ctx.enter_context(tc.tile_pool(name="consts", bufs=1))
        xpool = ctx.enter_context(tc.tile_pool(name="xin", bufs=3))
        # PSUM pools: input-projection(+accumulated recurrence) chunks, double buffered
        pr_pool = ctx.enter_context(tc.tile_pool(name="pr", bufs=2, space="PSUM"))
        pz_pool = ctx.enter_context(tc.tile_pool(name="pz", bufs=2, space="PSUM"))
        pn_pool = ctx.enter_context(tc.tile_pool(name="pn", bufs=2, space="PSUM"))
        hp_pool = ctx.enter_context(tc.tile_pool(name="hpn", bufs=2, space="PSUM"))
        work = ctx.enter_context(tc.tile_pool(name="work", bufs=4))
        ring_pool = ctx.enter_context(tc.tile_pool(name="ring", bufs=2))

        w_ih = consts.tile([IG, 3 * HG], f32)
        w_hh = consts.tile([HG, 3 * HG], f32)
        bias4 = consts.tile([HG, 4], f32)
        nc.sync.dma_start(out=w_ih, in_=wih[:, :])
        nc.sync.dma_start(out=w_hh, in_=whh[:, :])
        nc.sync.dma_start(out=bias4, in_=biases[:, :])
        bias_r = bias4[:, 0:1]
        bias_z = bias4[:, 1:2]
        b_ihn = bias4[:, 2:3]
        b_hhn = bias4[:, 3:4]

        h_init = consts.tile([HG, B], f32)
        nc.vector.memset(h_init, 0.0)

        n_chunks = (T + PSUM_STEPS - 1) // PSUM_STEPS

        h_prev = h_init
        ring = None
        for c in range(n_chunks):
            t0 = c * PSUM_STEPS
            steps = min(PSUM_STEPS, T - t0)
            nb = steps * B

            x_c = xpool.tile([IG, PSUM_STEPS * B], f32, tag="xc")
            nc.sync.dma_start(out=x_c[:, :nb], in_=xT[:, t0 * B : t0 * B + nb])

            p_r = pr_pool.tile([HG, PSUM_STEPS * B], f32, tag="pr")
            p_z = pz_pool.tile([HG, PSUM_STEPS * B], f32, tag="pz")
            p_n = pn_pool.tile([HG, PSUM_STEPS * B], f32, tag="pn")
            # input projections for the whole chunk: xp_j^T [HG, (t,b)]
            nc.tensor.matmul(p_r[:, :nb], w_ih[:, 0:HG], x_c[:, :nb],
                             start=True, stop=False, skip_group_check=True)
            nc.tensor.matmul(p_z[:, :nb], w_ih[:, HG:2 * HG], x_c[:, :nb],
                             start=True, stop=False, skip_group_check=True)
            nc.tensor.matmul(p_n[:, :nb], w_ih[:, 2 * HG:3 * HG], x_c[:, :nb],
                             start=True, stop=True, skip_group_check=True)
            xn_sb = xpool.tile([HG, PSUM_STEPS * B], f32, tag="xnsb")
            nc.scalar.activation(xn_sb[:, :nb], p_n[:, :nb], AF.Copy)

            for s in range(steps):
                t = t0 + s
                sl = slice(s * B, (s + 1) * B)
                if t % RING_STEPS == 0:
                    ring = ring_pool.tile([HG, RING_STEPS * B], f32, tag="ring")
                rsl = slice((t % RING_STEPS) * B, (t % RING_STEPS + 1) * B)

                # Recurrence matmuls, split by linearity:
                #   h_{t-1} = n + zh + zn'   (the three parts of the previous
                #   step's GRU update), each fed to PE as soon as available so
                #   the chain tail is only  tanh -> zn' -> matmul.
                hpn = hp_pool.tile([HG, B], f32, tag="hpn")
                if t > 0:
                    first_n = True
                    for rhs in prev_parts:
                        nc.tensor.matmul(p_r[:, sl], w_hh[:, 0:HG], rhs,
                                         start=False, stop=True,
                                         skip_group_check=True)
                        nc.tensor.matmul(p_z[:, sl], w_hh[:, HG:2 * HG], rhs,
                                         start=False, stop=True,
                                         skip_group_check=True)
                        nc.tensor.matmul(hpn, w_hh[:, 2 * HG:3 * HG], rhs,
                                         start=first_n, stop=True,
                                         skip_group_check=True)
                        first_n = False
                else:
                    # h_{-1} = 0: hp contribution is zero; just clear hpn
                    nc.tensor.matmul(hpn, w_hh[:, 2 * HG:3 * HG], h_init,
                                     start=True, stop=True,
                                     skip_group_check=True)

                r_sb = work.tile([HG, B], f32, tag="r")
                z_sb = work.tile([HG, B], f32, tag="z")
                nc.scalar.activation(r_sb, p_r[:, sl], AF.Sigmoid, bias=bias_r)
                nc.scalar.activation(z_sb, p_z[:, sl], AF.Sigmoid, bias=bias_z)

                # u = (hpn + b_hhn) * r
                u = work.tile([HG, B], f32, tag="u")
                nc.vector.scalar_tensor_tensor(
                    out=u, in0=hpn, scalar=b_hhn, in1=r_sb,
                    op0=ALU.add, op1=ALU.mult)
                # n_arg = (xpn + b_ihn) + u
                n_arg = work.tile([HG, B], f32, tag="narg")
                nc.vector.scalar_tensor_tensor(
                    out=n_arg, in0=xn_sb[:, sl], scalar=b_ihn, in1=u,
                    op0=ALU.add, op1=ALU.add)
                # zh = z * h_prev (off critical path, on GPSIMD)
                zh = work.tile([HG, B], f32, tag="zh")
                nc.gpsimd.tensor_mul(zh, z_sb, h_prev)
                n_sb = work.tile([HG, B], f32, tag="n")
                nc.scalar.activation(n_sb, n_arg, AF.Tanh)
                # zn' = -(n * z)   -- the only post-tanh op on the chain
                znm = work.tile([HG, B], f32, tag="znm")
                nc.vector.scalar_tensor_tensor(
                    out=znm, in0=n_sb, scalar=-1.0, in1=z_sb,
                    op0=ALU.mult, op1=ALU.mult)
                prev_parts = (zh, n_sb, znm)

                # h_new = n + zh + zn'  (output only; GPSIMD, off chain)
                w1 = work.tile([HG, B], f32, tag="w1")
                nc.gpsimd.tensor_add(w1, n_sb, zh)
                h_new = ring[:, rsl]
                nc.gpsimd.tensor_add(h_new, w1, znm)
                h_prev = h_new

                if (t + 1) % RING_STEPS == 0:
                    base = (t + 1 - RING_STEPS) * B
                    nc.sync.dma_start(out=y[:, base : base + RING_STEPS * B],
                                      in_=ring)
    nc.finalize()
    return nc


def _get_program():
    if "nc" not in _CACHE:
        _CACHE["nc"] = _build_program()
    return _CACHE["nc"]


def _prep_inputs(x, W_ih, W_hh, b_ih, b_hh):
    x = np.asarray(x, dtype=np.float32)
    W_ih = np.asarray(W_ih, dtype=np.float32)
    W_hh = np.asarray(W_hh, dtype=np.float32)
    b_ih = np.asarray(b_ih, dtype=np.float32)
    b_hh = np.asarray(b_hh, dtype=np.float32)

    # x [B,T,IN] -> per group [IG, T*B] with free index = t*B + b
    xg = x.reshape(B, T, G, IG)
    xT = np.ascontiguousarray(np.transpose(xg, (2, 3, 1, 0))).reshape(G, IG, T * B)

    wihT = np.ascontiguousarray(np.transpose(W_ih, (0, 2, 1)))  # [G, IG, 3HG]
    whhT = np.ascontiguousarray(np.transpose(W_hh, (0, 2, 1)))  # [G, HG, 3HG]

    biases = np.empty((G, HG, 4), np.float32)
    biases[:, :, 0] = b_ih[:, 0:HG] + b_hh[:, 0:HG]           # r
    biases[:, :, 1] = b_ih[:, HG:2 * HG] + b_hh[:, HG:2 * HG]  # z
    biases[:, :, 2] = b_ih[:, 2 * HG:3 * HG]                   # b_ihn
    biases[:, :, 3] = b_hh[:, 2 * HG:3 * HG]                   # b_hhn

    in_maps = []
    for g in range(G):
        in_maps.append({
            "xT": xT[g],
            "wih": wihT[g],
            "whh": whhT[g],
            "biases": biases[g],
        })
    return in_maps


def _assemble(results):
    out = np.empty((B, T, HID), np.float32)
    for g in range(G):
        yg = results[g]["y"].reshape(HG, T, B)          # [h, t, b]
        out[:, :, g * HG:(g + 1) * HG] = np.transpose(yg, (2, 1, 0))
    return out


def run(x, W_ih, W_hh, b_ih, b_hh, trace=False):
    from concourse.bass_utils import run_bass_kernel_spmd

    nc = _get_program()
    in_maps = _prep_inputs(x, W_ih, W_hh, b_ih, b_hh)
    res = run_bass_kernel_spmd(nc, in_maps, list(range(G)), trace=trace)
    return _assemble(res.results), res


def kernel(x, W_ih, W_hh, b_ih, b_hh):
    out, _ = run(x, W_ih, W_hh, b_ih, b_hh)
    return out



# revision 3
# speedup vs baseline: 1.0455x; 1.0455x over previous
# Grouped GRU layer on 8 Trainium2 NeuronCores (one group per core).
#
# Problem: x [64, 500, 1024], 8 independent groups of (IG=128 -> HG=128) GRUs.
#   xp = einsum('btgi,gji->btgj', xg, W_ih) + b_ih        (input projection)
#   per step: hp = h @ W_hh[g].T + b_hh
#             r = sig(xr+hr); z = sig(xz+hz); n = tanh(xn + r*hn)
#             h = (1-z)*n + z*h
#
# Sharding: group g -> core g. Per-core layout fully "transposed":
#   state h^T [HG=128 partitions, B=64 free], weights pre-transposed on host.
#
# v2: the per-step serial chain is the wall (T=500 dependent steps). Changes
# vs v1: bf16 recurrent matmuls (4x fewer PE cycles/row), 2-part state
# decomposition h = zh + w with zh = z*h_prev (ready early) and
# w = (1-z)*n (the only post-tanh op on the chain), sigmoid(r) computed
# in-place in PSUM (cheaper ACT access), per-partition biases folded into
# ACT bias operands / stt scalars, and the xn chunk copy scheduled into ACT
# queue slack of the previous chunk.

import numpy as np

B, T, IN, HID, G = 64, 500, 1024, 1024, 8
IG, HG = 128, 128

PSUM_STEPS = 8          # steps per PSUM bank chunk ([128, 8*64] fp32 = 1 bank)
RING_STEPS = 50         # output ring buffer length (steps) per DMA-out chunk

_CACHE = {}


def _build_program():
    import concourse.tile as tile
    from concourse import bacc, mybir

    f32 = mybir.dt.float32
    bf16 = mybir.dt.bfloat16
    AF = mybir.ActivationFunctionType
    ALU = mybir.AluOpType

    nc = bacc.Bacc()
    xT = nc.declare_dram_parameter("xT", [IG, T * B], bf16, isOutput=False)
    wih = nc.declare_dram_parameter("wih", [IG, 3 * HG], bf16, isOutput=False)
    whh = nc.declare_dram_parameter("whh", [HG, 3 * HG], bf16, isOutput=False)
    # per-partition bias columns: [r_bias, z_bias, b_ihn, b_hhn]
    biases = nc.declare_dram_parameter("biases", [HG, 4], f32, isOutput=False)
    y = nc.declare_dram_parameter("y", [HG, T * B], f32, isOutput=True)

    from contextlib import ExitStack

    with tile.TileContext(nc) as tc, ExitStack() as ctx:
        consts = ctx.enter_context(tc.tile_pool(name="consts", bufs=1))
        xpool = ctx.enter_context(tc.tile_pool(name="xin", bufs=3))
        xnpool = ctx.enter_context(tc.tile_pool(name="xnsb", bufs=2))
        pr_pool = ctx.enter_context(tc.tile_pool(name="pr", bufs=2, space="PSUM"))
        pz_pool = ctx.enter_context(tc.tile_pool(name="pz", bufs=2, space="PSUM"))
        pn_pool = ctx.enter_context(tc.tile_pool(name="pn", bufs=2, space="PSUM"))
        hp_pool = ctx.enter_context(tc.tile_pool(name="hpn", bufs=2, space="PSUM"))
        work = ctx.enter_context(tc.tile_pool(name="work", bufs=4))
        ring_pool = ctx.enter_context(tc.tile_pool(name="ring", bufs=2))

        w_ih = consts.tile([IG, 3 * HG], bf16)
        w_hh = consts.tile([HG, 3 * HG], bf16)
        bias4 = consts.tile([HG, 4], f32)
        nc.sync.dma_start(out=w_ih, in_=wih[:, :])
        nc.sync.dma_start(out=w_hh, in_=whh[:, :])
        nc.sync.dma_start(out=bias4, in_=biases[:, :])
        bias_r = bias4[:, 0:1]
        bias_z = bias4[:, 1:2]
        b_ihn = bias4[:, 2:3]
        b_hhn = bias4[:, 3:4]

        zeros_sb = consts.tile([HG, B], f32)
        nc.vector.memset(zeros_sb, 0.0)
        h_init = consts.tile([HG, B], f32)
        nc.vector.memset(h_init, 0.0)

        n_chunks = (T + PSUM_STEPS - 1) // PSUM_STEPS
        CB = PSUM_STEPS * B

        # chunk-level state carried across the step loop
        h_prev = h_init
        ring = None
        cur = {}   # current chunk PSUM tiles / xn_sb
        nxt = {}   # next chunk tiles (prefetched)

        def start_chunk(c):
            """DMA + input projections + xn copy issue for chunk c."""
            t0 = c * PSUM_STEPS
            steps = min(PSUM_STEPS, T - t0)
            nb = steps * B
            d = {}
            d["steps"] = steps
            d["t0"] = t0
            x_c = xpool.tile([IG, CB], bf16, tag="xc")
            nc.sync.dma_start(out=x_c[:, :nb], in_=xT[:, t0 * B : t0 * B + nb])
            p_r = pr_pool.tile([HG, CB], f32, tag="pr")
            p_z = pz_pool.tile([HG, CB], f32, tag="pz")
            p_n = pn_pool.tile([HG, CB], f32, tag="pn")
            nc.tensor.matmul(p_r[:, :nb], w_ih[:, 0:HG], x_c[:, :nb],
                             start=True, stop=False, skip_group_check=True)
            nc.tensor.matmul(p_z[:, :nb], w_ih[:, HG:2 * HG], x_c[:, :nb],
                             start=True, stop=False, skip_group_check=True)
            nc.tensor.matmul(p_n[:, :nb], w_ih[:, 2 * HG:3 * HG], x_c[:, :nb],
                             start=True, stop=True, skip_group_check=True)
            d["p_r"], d["p_z"], d["p_n"] = p_r, p_z, p_n
            # xn -> SBUF copy, split in two so each half fits in ACT queue
            # slack between chain ops
            xn_sb = xnpool.tile([HG, CB], f32, tag="xnsb")
            d["xn_sb"] = xn_sb
            d["xn_copied"] = 0
            return d

        def copy_half_xn(d, half):
            nb = d["steps"] * B
            lo = half * (CB // 2)
            hi = min((half + 1) * (CB // 2), nb)
            if lo < hi:
                nc.scalar.activation(d["xn_sb"][:, lo:hi], d["p_n"][:, lo:hi],
                                     AF.Copy)

        cur = start_chunk(0)
        copy_half_xn(cur, 0)
        copy_half_xn(cur, 1)

        for c in range(n_chunks):
            steps = cur["steps"]
            t0 = cur["t0"]
            p_r, p_z = cur["p_r"], cur["p_z"]
            xn_sb = cur["xn_sb"]

            for s in range(steps):
                t = t0 + s
                sl = slice(s * B, (s + 1) * B)
                if t % RING_STEPS == 0:
                    ring = ring_pool.tile([HG, RING_STEPS * B], f32, tag="ring")
                rsl = slice((t % RING_STEPS) * B, (t % RING_STEPS + 1) * B)

                # --- gates of step t (pre-activations already in PSUM) ---
                # sigmoid(r) -> SBUF (DVE may read only ONE PSUM operand, and
                # u must read hpn from PSUM)
                r_sb = work.tile([HG, B], f32, tag="r")
                nc.scalar.activation(r_sb, p_r[:, sl], AF.Sigmoid, bias=bias_r)

                # u = (hpn + b_hhn) * r    (chain)
                u = work.tile([HG, B], f32, tag="u")
                hpn_in = cur.get("hpn", None) if s == 0 else hpn
                if t == 0:
                    hpn_in = zeros_sb
                nc.vector.scalar_tensor_tensor(
                    out=u, in0=hpn_in, scalar=b_hhn, in1=r_sb,
                    op0=ALU.add, op1=ALU.mult)

                # z path (off the critical chain)
                z_sb = work.tile([HG, B], bf16, tag="z")
                nc.scalar.activation(z_sb, p_z[:, sl], AF.Sigmoid, bias=bias_z)
                zm1 = work.tile([HG, B], bf16, tag="zm1")
                nc.gpsimd.tensor_scalar_sub(zm1, z_sb, 1.0)
                zh = work.tile([HG, B], bf16, tag="zh")
                nc.gpsimd.tensor_mul(zh, z_sb, h_prev)

                # n_arg = (u + b_ihn) + xn  (chain)
                n_arg = work.tile([HG, B], f32, tag="narg")
                nc.vector.scalar_tensor_tensor(
                    out=n_arg, in0=u, scalar=b_ihn, in1=xn_sb[:, sl],
                    op0=ALU.add, op1=ALU.add)
                n_sb = work.tile([HG, B], bf16, tag="n")
                nc.scalar.activation(n_sb, n_arg, AF.Tanh)
                # w = (1-z)*n = (-n) * (z-1)   (the only post-tanh chain op)
                w_sb = work.tile([HG, B], bf16, tag="w")
                nc.vector.scalar_tensor_tensor(
                    out=w_sb, in0=n_sb, scalar=-1.0, in1=zm1,
                    op0=ALU.mult, op1=ALU.mult)

                # h_new = zh + w (off chain; for output + next-step zh)
                h_new = ring[:, rsl]
                nc.gpsimd.tensor_add(h_new, zh, w_sb)
                h_prev = h_new

                # --- recurrence matmuls feeding step t+1 ---
                if t + 1 < T:
                    in_this = s + 1 < steps
                    if in_this:
                        sl1 = slice((s + 1) * B, (s + 2) * B)
                        t_r, t_z = p_r[:, sl1], p_z[:, sl1]
                    else:
                        # next chunk: prefetch DMA + projections first
                        nxt = start_chunk(c + 1)
                        sl1 = slice(0, B)
                        t_r, t_z = nxt["p_r"][:, sl1], nxt["p_z"][:, sl1]
                    hpn = hp_pool.tile([HG, B], f32, tag="hpn")
                    # zh-part (ready before tanh; runs in PE idle window)
                    nc.tensor.matmul(t_r, w_hh[:, 0:HG], zh,
                                     start=False, stop=False,
                                     skip_group_check=True)
                    nc.tensor.matmul(t_z, w_hh[:, HG:2 * HG], zh,
                                     start=False, stop=False,
                                     skip_group_check=True)
                    nc.tensor.matmul(hpn, w_hh[:, 2 * HG:3 * HG], zh,
                                     start=True, stop=False,
                                     skip_group_check=True)
                    # w-part: r-gate first (unblocks sigmoid), then n, then z
                    nc.tensor.matmul(t_r, w_hh[:, 0:HG], w_sb,
                                     start=False, stop=True,
                                     skip_group_check=True)
                    nc.tensor.matmul(hpn, w_hh[:, 2 * HG:3 * HG], w_sb,
                                     start=False, stop=True,
                                     skip_group_check=True)
                    nc.tensor.matmul(t_z, w_hh[:, HG:2 * HG], w_sb,
                                     start=False, stop=True,
                                     skip_group_check=True)
                    if not in_this:
                        nxt["hpn"] = hpn

                # xn copies for the *next* chunk go into ACT slack of steps
                # 4 and 5 (projections for chunk c+1 are issued at s ==
                # steps-1, so for timing they land during the early steps of
                # chunk c+1 -- copy halves at s==0/1 of the new chunk instead)
                if s == 1 and c > 0:
                    copy_half_xn(cur, 1)

                if (t + 1) % RING_STEPS == 0:
                    base = (t + 1 - RING_STEPS) * B
                    nc.sync.dma_start(out=y[:, base : base + RING_STEPS * B],
                                      in_=ring)

            if c + 1 < n_chunks:
                # first xn half right away (needed by step t0+0 of chunk c+1)
                copy_half_xn(nxt, 0)
                cur = nxt
    nc.finalize()
    return nc


def _get_program():
    if "nc" not in _CACHE:
        _CACHE["nc"] = _build_program()
    return _CACHE["nc"]


def _prep_inputs(x, W_ih, W_hh, b_ih, b_hh):
    import ml_dtypes

    bf = ml_dtypes.bfloat16
    x = np.asarray(x, dtype=np.float32)
    W_ih = np.asarray(W_ih, dtype=np.float32)
    W_hh = np.asarray(W_hh, dtype=np.float32)
    b_ih = np.asarray(b_ih, dtype=np.float32)
    b_hh = np.asarray(b_hh, dtype=np.float32)

    # x [B,T,IN] -> per group [IG, T*B] with free index = t*B + b
    xg = x.reshape(B, T, G, IG)
    xT = np.ascontiguousarray(np.transpose(xg, (2, 3, 1, 0))).reshape(G, IG, T * B)

    wihT = np.ascontiguousarray(np.transpose(W_ih, (0, 2, 1)))  # [G, IG, 3HG]
    whhT = np.ascontiguousarray(np.transpose(W_hh, (0, 2, 1)))  # [G, HG, 3HG]

    biases = np.empty((G, HG, 4), np.float32)
    biases[:, :, 0] = b_ih[:, 0:HG] + b_hh[:, 0:HG]           # r
    biases[:, :, 1] = b_ih[:, HG:2 * HG] + b_hh[:, HG:2 * HG]  # z
    biases[:, :, 2] = b_ih[:, 2 * HG:3 * HG]                   # b_ihn
    biases[:, :, 3] = b_hh[:, 2 * HG:3 * HG]                   # b_hhn

    in_maps = []
    for g in range(G):
        in_maps.append({
            "xT": xT[g].astype(bf),
            "wih": wihT[g].astype(bf),
            "whh": whhT[g].astype(bf),
            "biases": biases[g],
        })
    return in_maps


def _assemble(results):
    out = np.empty((B, T, HID), np.float32)
    for g in range(G):
        yg = results[g]["y"].reshape(HG, T, B)          # [h, t, b]
        out[:, :, g * HG:(g + 1) * HG] = np.transpose(yg, (2, 1, 0))
    return out


def run(x, W_ih, W_hh, b_ih, b_hh, trace=False):
    from concourse.bass_utils import run_bass_kernel_spmd

    nc = _get_program()
    in_maps = _prep_inputs(x, W_ih, W_hh, b_ih, b_hh)
    res = run_bass_kernel_spmd(nc, in_maps, list(range(G)), trace=trace)
    return _assemble(res.results), res


def kernel(x, W_ih, W_hh, b_ih, b_hh):
    out, _ = run(x, W_ih, W_hh, b_ih, b_hh)
    return out


# revision 15
# speedup vs baseline: 1.2948x; 1.2384x over previous
# Grouped GRU layer on 8 Trainium2 NeuronCores (one group per core).
#
# Problem: x [64, 500, 1024], 8 independent groups of (IG=128 -> HG=128) GRUs.
#   xp = einsum('btgi,gji->btgj', xg, W_ih) + b_ih        (input projection)
#   per step: hp = h @ W_hh[g].T + b_hh
#             r = sig(xr+hr); z = sig(xz+hz); n = tanh(xn + r*hn)
#             h = (1-z)*n + z*h
#
# Sharding: group g -> core g. Per-core layout fully "transposed":
#   state h^T [HG=128 partitions, B=64 free], weights pre-transposed on host.
#
# The per-step serial dependency chain is the wall (T=500 steps); the kernel
# minimizes the number and cost of chained engine visits per step:
#   sigmoid(r) [ACT] -> scan [DVE] -> tanh [ACT] -> w=omz*n [DVE] -> matmul
# - bf16 recurrent matmuls (1 PE cycle/row instead of 4)
# - W_z/b_z negated on host: sigmoid directly yields omz = 1-z, and the
#   post-tanh multiply w = omz*n has both producers on ACT (single semaphore,
#   engine-level wait)
# - state fed to PE as three parts h = h_prev + q + w (q = -omz*h_prev), so
#   only w is tanh-dependent
# - u = r*(hn+b_hhn) and n_arg = u + xn fused into ONE tensor_tensor_scan
#   over interleaved lanes: state(2b) = hn_b, state(2b+1) = r_b*hn_b + xn_b;
#   hn (+b_hhn) and xn are staged into the interleaved buffer off-chain
# - xn chunk copies (with b_ihn folded) scheduled into ACT queue slack

import numpy as np

B, T, IN, HID, G = 64, 500, 1024, 1024, 8
IG, HG = 128, 128

PSUM_STEPS = 8          # steps per PSUM bank chunk ([128, 8*64] fp32 = 1 bank)
RING_STEPS = 50         # output ring buffer length (steps) per DMA-out chunk

_CACHE = {}


def _build_program():
    import concourse.tile as tile
    from concourse import bacc, mybir

    f32 = mybir.dt.float32
    bf16 = mybir.dt.bfloat16
    AF = mybir.ActivationFunctionType
    ALU = mybir.AluOpType

    nc = bacc.Bacc()
    xT = nc.declare_dram_parameter("xT", [IG, T * B], bf16, isOutput=False)
    wih = nc.declare_dram_parameter("wih", [IG, 3 * HG], bf16, isOutput=False)
    whh = nc.declare_dram_parameter("whh", [HG, 3 * HG], bf16, isOutput=False)
    # per-partition bias columns: [r_bias, -z_bias, b_ihn, b_hhn]
    biases = nc.declare_dram_parameter("biases", [HG, 4], f32, isOutput=False)
    y = nc.declare_dram_parameter("y", [HG, T * B], f32, isOutput=True)

    from contextlib import ExitStack

    with tile.TileContext(nc) as tc, ExitStack() as ctx:
        consts = ctx.enter_context(tc.tile_pool(name="consts", bufs=1))
        xpool = ctx.enter_context(tc.tile_pool(name="xin", bufs=3))
        pr_pool = ctx.enter_context(tc.tile_pool(name="pr", bufs=2, space="PSUM"))
        pz_pool = ctx.enter_context(tc.tile_pool(name="pz", bufs=2, space="PSUM"))
        pn_pool = ctx.enter_context(tc.tile_pool(name="pn", bufs=2, space="PSUM"))
        hp_pool = ctx.enter_context(tc.tile_pool(name="hpn", bufs=2, space="PSUM"))
        work = ctx.enter_context(tc.tile_pool(name="work", bufs=4))
        ring_pool = ctx.enter_context(tc.tile_pool(name="ring", bufs=2))

        w_ih = consts.tile([IG, 3 * HG], bf16)
        w_hh = consts.tile([HG, 3 * HG], bf16)
        bias4 = consts.tile([HG, 4], f32)
        nc.sync.dma_start(out=w_ih, in_=wih[:, :])
        nc.sync.dma_start(out=w_hh, in_=whh[:, :])
        nc.sync.dma_start(out=bias4, in_=biases[:, :])
        bias_r = bias4[:, 0:1]
        bias_zn = bias4[:, 1:2]   # negated z bias (W_z also negated on host)
        b_ihn = bias4[:, 2:3]
        b_hhn = bias4[:, 3:4]

        zeros_sb = consts.tile([HG, B], f32)
        nc.vector.memset(zeros_sb, 0.0)

        n_chunks = (T + PSUM_STEPS - 1) // PSUM_STEPS
        CB = PSUM_STEPS * B

        # Interleaved scan operand buffers, one pair per chunk parity.
        # mix0: even lanes 0 (reset state to hn), odd lanes r (sigmoid out).
        # mix1: even lanes hn+b_hhn (staged per step), odd lanes xn+b_ihn
        # (chunk copy). Even lanes of mix0 are memset once and never touched.
        mix0 = [consts.tile([HG, 2 * CB], f32, name=f"mix0_{i}") for i in range(2)]
        mix1 = [consts.tile([HG, 2 * CB], f32, name=f"mix1_{i}") for i in range(2)]
        for mt in mix0 + mix1:
            nc.vector.memset(mt, 0.0)
        # t=0: hn lanes of slice 0 must hold plain b_hhn (hpn(0) = 0)
        nc.scalar.activation(mix1[0][:, 0:2 * B:2], zeros_sb,
                             AF.Identity, bias=b_hhn)

        h_prev = zeros_sb
        ring = None
        nxt = {}

        def start_chunk(c):
            """DMA + input projections for chunk c."""
            t0 = c * PSUM_STEPS
            steps = min(PSUM_STEPS, T - t0)
            nb = steps * B
            d = {"steps": steps, "t0": t0, "par": c % 2}
            x_c = xpool.tile([IG, CB], bf16, tag="xc")
            nc.sync.dma_start(out=x_c[:, :nb], in_=xT[:, t0 * B : t0 * B + nb])
            p_r = pr_pool.tile([HG, CB], f32, tag="pr")
            p_z = pz_pool.tile([HG, CB], f32, tag="pz")
            p_n = pn_pool.tile([HG, CB], f32, tag="pn")
            nc.tensor.matmul(p_r[:, :nb], w_ih[:, 0:HG], x_c[:, :nb],
                             start=True, stop=False, skip_group_check=True)
            nc.tensor.matmul(p_z[:, :nb], w_ih[:, HG:2 * HG], x_c[:, :nb],
                             start=True, stop=False, skip_group_check=True)
            nc.tensor.matmul(p_n[:, :nb], w_ih[:, 2 * HG:3 * HG], x_c[:, :nb],
                             start=True, stop=True, skip_group_check=True)
            d["p_r"], d["p_z"], d["p_n"] = p_r, p_z, p_n
            return d

        def copy_half_xn(d, half):
            # xn -> odd lanes of mix1 with b_ihn folded in
            nb = d["steps"] * B
            lo = half * (CB // 2)
            hi = min((half + 1) * (CB // 2), nb)
            if lo < hi:
                m1 = mix1[d["par"]]
                nc.scalar.activation(m1[:, 2 * lo + 1 : 2 * hi : 2],
                                     d["p_n"][:, lo:hi],
                                     AF.Identity, bias=b_ihn)

        cur = start_chunk(0)
        copy_half_xn(cur, 0)
        copy_half_xn(cur, 1)

        for c in range(n_chunks):
            steps = cur["steps"]
            t0 = cur["t0"]
            p_r, p_z = cur["p_r"], cur["p_z"]
            m0c, m1c = mix0[cur["par"]], mix1[cur["par"]]

            for s in range(steps):
                t = t0 + s
                sl = slice(s * B, (s + 1) * B)
                msl = slice(2 * s * B, 2 * (s + 1) * B)
                if t % RING_STEPS == 0:
                    ring = ring_pool.tile([HG, RING_STEPS * B], f32, tag="ring")
                rsl = slice((t % RING_STEPS) * B, (t % RING_STEPS + 1) * B)

                # --- step t gates (pre-activations in PSUM) ---
                # sigmoid(r) into the odd lanes of mix0
                nc.scalar.activation(m0c[:, 2 * s * B + 1 : 2 * (s + 1) * B : 2],
                                     p_r[:, sl], AF.Sigmoid, bias=bias_r)

                # fused u+n_arg: state(2b) = hn_b, state(2b+1) = r_b*hn_b+xn_b
                sc = work.tile([HG, 2 * B], f32, tag="sc")
                nc.vector.tensor_tensor_scan(sc, m0c[:, msl], m1c[:, msl],
                                             0.0, ALU.mult, ALU.add)

                # omz = 1 - z = sigmoid(-a_z)
                omz = work.tile([HG, B], bf16, tag="omz")
                nc.scalar.activation(omz, p_z[:, sl], AF.Sigmoid, bias=bias_zn)
                # q = -omz * h_prev   (part 2 of next state; off chain)
                q_sb = work.tile([HG, B], bf16, tag="q")
                nc.vector.scalar_tensor_tensor(
                    out=q_sb, in0=omz, scalar=-1.0, in1=h_prev,
                    op0=ALU.mult, op1=ALU.mult)

                n_sb = work.tile([HG, B], bf16, tag="n")
                nc.scalar.activation(n_sb, sc[:, 1::2], AF.Tanh)
                # w = omz * n   (the only post-tanh chain op)
                w_sb = work.tile([HG, B], bf16, tag="w")
                nc.vector.tensor_mul(w_sb, n_sb, omz)

                # h_new = h_prev + q + w. h_new on DVE so that w's tile has no
                # Pool reader (keeps w's attached wait on the tanh semaphore).
                w1 = work.tile([HG, B], f32, tag="w1")
                nc.gpsimd.tensor_add(w1, h_prev if t > 0 else zeros_sb, q_sb)
                h_new = ring[:, rsl]
                nc.vector.tensor_add(h_new, w1, w_sb)
                # bf16 copy of h for the next step's PE rhs
                h_bf = work.tile([HG, B], bf16, tag="hbf")
                nc.gpsimd.tensor_scalar_add(h_bf, h_new, 0.0)

                # --- recurrence matmuls feeding step t+1 ---
                if t + 1 < T:
                    in_this = s + 1 < steps
                    if in_this:
                        sl1 = slice((s + 1) * B, (s + 2) * B)
                        t_r, t_z = p_r[:, sl1], p_z[:, sl1]
                        m1n = m1c
                        e0 = 2 * (s + 1) * B
                    else:
                        nxt = start_chunk(c + 1)
                        t_r, t_z = nxt["p_r"][:, 0:B], nxt["p_z"][:, 0:B]
                        m1n = mix1[nxt["par"]]
                        e0 = 0
                    hpn = hp_pool.tile([HG, B], f32, tag="hpn")
                    # h_prev-part (ready at h_bf of step t-1; earliest)
                    if t > 0:
                        nc.tensor.matmul(t_r, w_hh[:, 0:HG], h_bf_prev,
                                         start=False, stop=False,
                                         skip_group_check=True)
                        nc.tensor.matmul(t_z, w_hh[:, HG:2 * HG], h_bf_prev,
                                         start=False, stop=False,
                                         skip_group_check=True)
                        nc.tensor.matmul(hpn, w_hh[:, 2 * HG:3 * HG], h_bf_prev,
                                         start=True, stop=False,
                                         skip_group_check=True)
                    # q-part (ready after omz, mid-step)
                    nc.tensor.matmul(t_r, w_hh[:, 0:HG], q_sb,
                                     start=False, stop=False,
                                     skip_group_check=True)
                    nc.tensor.matmul(t_z, w_hh[:, HG:2 * HG], q_sb,
                                     start=False, stop=False,
                                     skip_group_check=True)
                    nc.tensor.matmul(hpn, w_hh[:, 2 * HG:3 * HG], q_sb,
                                     start=(t == 0), stop=False,
                                     skip_group_check=True)
                    # w-part: r-gate first (unblocks sigmoid), then n, then z
                    nc.tensor.matmul(t_r, w_hh[:, 0:HG], w_sb,
                                     start=False, stop=True,
                                     skip_group_check=True)
                    nc.tensor.matmul(hpn, w_hh[:, 2 * HG:3 * HG], w_sb,
                                     start=False, stop=True,
                                     skip_group_check=True)
                    nc.tensor.matmul(t_z, w_hh[:, HG:2 * HG], w_sb,
                                     start=False, stop=True,
                                     skip_group_check=True)
                    # stage hn+b_hhn into the even lanes of the next step's
                    # scan buffer
                    nc.vector.tensor_scalar_add(
                        m1n[:, e0 : e0 + 2 * B : 2], hpn, b_hhn)

                h_prev = h_new
                h_bf_prev = h_bf

                if s == 1 and c > 0:
                    copy_half_xn(cur, 1)

                if (t + 1) % RING_STEPS == 0:
                    base = (t + 1 - RING_STEPS) * B
                    nc.sync.dma_start(out=y[:, base : base + RING_STEPS * B],
                                      in_=ring)

            if c + 1 < n_chunks:
                copy_half_xn(nxt, 0)
                cur = nxt
    nc.finalize()
    return nc


def _get_program():
    if "nc" not in _CACHE:
        _CACHE["nc"] = _build_program()
    return _CACHE["nc"]


def _prep_inputs(x, W_ih, W_hh, b_ih, b_hh):
    import ml_dtypes

    bf = ml_dtypes.bfloat16
    x = np.asarray(x, dtype=np.float32)
    W_ih = np.asarray(W_ih, dtype=np.float32)
    W_hh = np.asarray(W_hh, dtype=np.float32)
    b_ih = np.asarray(b_ih, dtype=np.float32)
    b_hh = np.asarray(b_hh, dtype=np.float32)

    # x [B,T,IN] -> per group [IG, T*B] with free index = t*B + b
    xg = x.reshape(B, T, G, IG)
    xT = np.ascontiguousarray(np.transpose(xg, (2, 3, 1, 0))).reshape(G, IG, T * B)

    wihT = np.transpose(W_ih, (0, 2, 1)).copy()  # [G, IG, 3HG]
    whhT = np.transpose(W_hh, (0, 2, 1)).copy()  # [G, HG, 3HG]
    # negate the z-gate weights so sigmoid yields omz = 1-z directly
    wihT[:, :, HG:2 * HG] *= -1.0
    whhT[:, :, HG:2 * HG] *= -1.0

    biases = np.empty((G, HG, 4), np.float32)
    biases[:, :, 0] = b_ih[:, 0:HG] + b_hh[:, 0:HG]              # r
    biases[:, :, 1] = -(b_ih[:, HG:2 * HG] + b_hh[:, HG:2 * HG])  # -z
    biases[:, :, 2] = b_ih[:, 2 * HG:3 * HG]                      # b_ihn
    biases[:, :, 3] = b_hh[:, 2 * HG:3 * HG]                      # b_hhn

    in_maps = []
    for g in range(G):
        in_maps.append({
            "xT": xT[g].astype(bf),
            "wih": wihT[g].astype(bf),
            "whh": whhT[g].astype(bf),
            "biases": biases[g],
        })
    return in_maps


def _assemble(results):
    out = np.empty((B, T, HID), np.float32)
    for g in range(G):
        yg = results[g]["y"].reshape(HG, T, B)          # [h, t, b]
        out[:, :, g * HG:(g + 1) * HG] = np.transpose(yg, (2, 1, 0))
    return out


def run(x, W_ih, W_hh, b_ih, b_hh, trace=False):
    from concourse.bass_utils import run_bass_kernel_spmd

    nc = _get_program()
    in_maps = _prep_inputs(x, W_ih, W_hh, b_ih, b_hh)
    res = run_bass_kernel_spmd(nc, in_maps, list(range(G)), trace=trace)
    return _assemble(res.results), res


def kernel(x, W_ih, W_hh, b_ih, b_hh):
    out, _ = run(x, W_ih, W_hh, b_ih, b_hh)
    return out
